# revision 8
# baseline (speedup 1.0000x reference)
"""Trainium2 Bass kernel for nn_BinaryBlock (RSign -> scaled binary conv1d
(K=3, pad=1) -> bias -> RPReLU).

Full inputs in, full output out. Data-parallel over batch: 8 cores x 2 images.
Per-core shard layout: [128, L] fp32 where partition p = b_local*64 + channel.

Math (forward only; STE parts of the reference are identity in the forward):
    xb  = where(x >= alpha, 1, -1)
    wb  = sign(w) * scale                    (per out-channel scale)
    y   = conv1d(xb, wb, pad=1) + bias
    out = where(y > gamma, y - gamma + zeta, beta*(y - gamma) + zeta)

Device computation:
    H' = 2*(x >= alpha) in {0,2}  (DVE tensor_scalar is_ge,mult; bf16 out)
    pad/halo columns of H' are set to 1.0 so that with T = conv(H', sign(w)),
    conv(xb) = T - S where S[co] = sum(sign(w[co,:,:])) for EVERY output col.
    t' = y - gamma = scale*T + c1,  c1 = bias - gamma - scale*S
    out = (1-beta)*relu(t') + (beta*t' + zeta)
        ACT1: v = Relu(scale*T + c1)            [per-partition scale/bias]
        ACT2: q = Identity(beta*scale*T + beta*c1 + zeta)
        DVE : out = (v * (1-beta)) + q          [scalar_tensor_tensor]
All conv arithmetic is exact (integer-valued products/sums in fp32 PSUM).
"""

import sys

if "/opt/trn_rl_repo" not in sys.path:
    sys.path.insert(0, "/opt/trn_rl_repo")

import numpy as np
import ml_dtypes

import concourse.bacc as bacc
import concourse.mybir as mybir
import concourse.tile as tile
from concourse.bass_utils import run_bass_kernel_spmd

P = 128          # SBUF partitions = 2 images x 64 channels
CH = 64          # channels
KTAPS = 3        # conv taps
CHUNK = 512      # PSUM bank = 512 fp32 -> matmul free dim
TW = 2048        # output columns per tile (4 PSUM banks)
L_FULL = 65536
N_CORES = 8
B_FULL = 16


def build_nc(
    L: int,
    tw: int = TW,
    repeats: int = 1,
    xbufs: int = 3,
    ebufs: int = 2,
    pbufs: int = 2,
):
    """Build the per-core Bass program for shard [P, L].

    repeats > 1 re-runs the whole body (idempotent) for marginal-cost timing.
    """
    assert L % tw == 0 and tw % CHUNK == 0
    n_tiles = L // tw
    assert n_tiles >= 2
    n_chunks = tw // CHUNK
    f32 = mybir.dt.float32
    bf16 = mybir.dt.bfloat16

    nc = bacc.Bacc("TRN2", target_bir_lowering=False, debug=False)
    x = nc.dram_tensor("x", [P, L], f32, kind="ExternalInput").ap()
    w = nc.dram_tensor("w", [KTAPS, P, P], bf16, kind="ExternalInput").ap()
    alpha2 = nc.dram_tensor("alpha2", [P, 1], f32, kind="ExternalInput").ap()
    relu_scale = nc.dram_tensor("relu_scale", [P, 1], f32, kind="ExternalInput").ap()
    relu_bias = nc.dram_tensor("relu_bias", [P, 1], f32, kind="ExternalInput").ap()
    id_scale = nc.dram_tensor("id_scale", [P, 1], f32, kind="ExternalInput").ap()
    id_bias = nc.dram_tensor("id_bias", [P, 1], f32, kind="ExternalInput").ap()
    ombeta = nc.dram_tensor("ombeta", [P, 1], f32, kind="ExternalInput").ap()
    y = nc.dram_tensor("y", [P, L], f32, kind="ExternalOutput").ap()

    xw = tw + 2  # input tile width incl. 1-col halo each side

    with tile.TileContext(nc) as tc:
        with (
            tc.tile_pool(name="const", bufs=1) as cpool,
            tc.tile_pool(name="xin", bufs=xbufs) as xpool,
            tc.tile_pool(name="hp", bufs=xbufs) as hpool,
            tc.tile_pool(name="eps", bufs=ebufs) as epool,
            tc.tile_pool(name="psum", bufs=pbufs, space="PSUM") as ppool,
        ):
            w_t = cpool.tile([P, KTAPS, P], bf16)
            for k in range(KTAPS):
                nc.sync.dma_start(out=w_t[:, k, :], in_=w[k])
            a_t = cpool.tile([P, 1], f32)
            rs_t = cpool.tile([P, 1], f32)
            rb_t = cpool.tile([P, 1], f32)
            is_t = cpool.tile([P, 1], f32)
            ib_t = cpool.tile([P, 1], f32)
            ob_t = cpool.tile([P, 1], f32)
            nc.sync.dma_start(out=a_t[:], in_=alpha2[:])
            nc.sync.dma_start(out=rs_t[:], in_=relu_scale[:])
            nc.sync.dma_start(out=rb_t[:], in_=relu_bias[:])
            nc.sync.dma_start(out=is_t[:], in_=id_scale[:])
            nc.sync.dma_start(out=ib_t[:], in_=id_bias[:])
            nc.sync.dma_start(out=ob_t[:], in_=ombeta[:])

            for i in range(n_tiles * repeats):
                i = i % n_tiles
                base = i * tw
                x_t = xpool.tile([P, xw], f32)
                h_t = hpool.tile([P, xw], bf16)
                # load x tile (halo col j maps to x col base-1+j), rsign it
                if i == 0:
                    nc.sync.dma_start(out=x_t[:, 1:xw], in_=x[:, 0 : tw + 1])
                    nc.vector.memset(h_t[:, 0:1], 1.0)
                    nc.vector.tensor_scalar(
                        out=h_t[:, 1:xw], in0=x_t[:, 1:xw],
                        scalar1=a_t[:], scalar2=2.0,
                        op0=mybir.AluOpType.is_ge, op1=mybir.AluOpType.mult,
                    )
                elif i == n_tiles - 1:
                    nc.sync.dma_start(out=x_t[:, 0 : xw - 1], in_=x[:, base - 1 : L])
                    nc.vector.memset(h_t[:, xw - 1 : xw], 1.0)
                    nc.vector.tensor_scalar(
                        out=h_t[:, 0 : xw - 1], in0=x_t[:, 0 : xw - 1],
                        scalar1=a_t[:], scalar2=2.0,
                        op0=mybir.AluOpType.is_ge, op1=mybir.AluOpType.mult,
                    )
                else:
                    nc.sync.dma_start(out=x_t[:], in_=x[:, base - 1 : base + tw + 1])
                    nc.vector.tensor_scalar(
                        out=h_t[:], in0=x_t[:],
                        scalar1=a_t[:], scalar2=2.0,
                        op0=mybir.AluOpType.is_ge, op1=mybir.AluOpType.mult,
                    )

                ps = ppool.tile([P, tw], f32)
                for c in range(n_chunks):
                    for k in range(KTAPS):
                        nc.tensor.matmul(
                            ps[:, c * CHUNK : (c + 1) * CHUNK],
                            w_t[:, k, :],
                            h_t[:, c * CHUNK + k : c * CHUNK + k + CHUNK],
                            start=(k == 0),
                            stop=(k == KTAPS - 1),
                        )

                v_t = epool.tile([P, tw], f32, tag="v")
                q_t = epool.tile([P, tw], f32, tag="q")
                o_t = epool.tile([P, tw], f32, tag="o")
                nc.scalar.activation(
                    out=v_t[:], in_=ps[:],
                    func=mybir.ActivationFunctionType.Relu,
                    bias=rb_t[:], scale=rs_t[:],
                )
                nc.scalar.activation(
                    out=q_t[:], in_=ps[:],
                    func=mybir.ActivationFunctionType.Identity,
                    bias=ib_t[:], scale=is_t[:],
                )
                nc.vector.scalar_tensor_tensor(
                    out=o_t[:], in0=v_t[:], scalar=ob_t[:], in1=q_t[:],
                    op0=mybir.AluOpType.mult, op1=mybir.AluOpType.add,
                )
                nc.sync.dma_start(out=y[:, base : base + tw], in_=o_t[:])
    nc.compile()
    return nc


def host_prep(alpha, weight, weight_scale, bias, beta, gamma, zeta):
    """Host-side parameter folding. Returns dict of small device inputs."""
    al = np.asarray(alpha, np.float32).reshape(CH)
    sc = np.asarray(weight_scale, np.float32).reshape(CH)
    bi = np.asarray(bias, np.float32).reshape(CH)
    be = np.asarray(beta, np.float32).reshape(CH)
    ga = np.asarray(gamma, np.float32).reshape(CH)
    ze = np.asarray(zeta, np.float32).reshape(CH)
    wgt = np.asarray(weight, np.float32)  # [CH, CH, KTAPS]

    sgn = np.sign(wgt).astype(np.float32)
    s_all = sgn.sum(axis=(1, 2)).astype(np.float32)  # [CH] integer-valued

    # Block-diagonal lhsT per tap: [p_in, p_out] with two [ci, co] blocks.
    w_np = np.zeros((KTAPS, P, P), dtype=ml_dtypes.bfloat16)
    for k in range(KTAPS):
        tk = sgn[:, :, k].T.astype(ml_dtypes.bfloat16)  # [ci, co]
        w_np[k, :CH, :CH] = tk
        w_np[k, CH:, CH:] = tk

    c1 = (bi - ga - sc * s_all).astype(np.float32)

    def vec(v):
        return np.tile(v.astype(np.float32), 2).reshape(P, 1)

    return {
        "w": w_np,
        "alpha2": vec(al),
        "relu_scale": vec(sc),
        "relu_bias": vec(c1),
        "id_scale": vec(be * sc),
        "id_bias": vec(be * c1 + ze),
        "ombeta": vec(1.0 - be),
    }


def kernel(x, alpha, weight, weight_scale, bias, beta, gamma, zeta):
    x = np.asarray(x, np.float32)
    B, Cin, L = x.shape
    assert (B, Cin, L) == (B_FULL, CH, L_FULL), (B, Cin, L)

    params = host_prep(alpha, weight, weight_scale, bias, beta, gamma, zeta)
    nc = build_nc(L)

    shards = np.ascontiguousarray(x.reshape(N_CORES, P, L))
    in_maps = [dict(params, x=shards[i]) for i in range(N_CORES)]
    res = run_bass_kernel_spmd(nc, in_maps, core_ids=list(range(N_CORES)))
    out = np.stack([res.results[i]["y"] for i in range(N_CORES)])
    return out.reshape(B, CH, L).astype(np.float32)


# revision 11
# speedup vs baseline: 1.4082x; 1.4082x over previous
"""Trainium2 Bass kernel for nn_BinaryBlock (RSign -> scaled binary conv1d
(K=3, pad=1) -> bias -> RPReLU).

Full inputs in, full output out. Data-parallel over batch: 8 cores x 2 images.
Per-core shard layout: [128, L] fp32 where partition p = b_local*64 + channel.

Math (forward only; STE parts of the reference are identity in the forward):
    xb  = where(x >= alpha, 1, -1)
    wb  = sign(w) * scale                    (per out-channel scale)
    y   = conv1d(xb, wb, pad=1) + bias
    out = where(y > gamma, y - gamma + zeta, beta*(y - gamma) + zeta)

Device computation:
    H' = 2*(x >= alpha) in {0,2}  (DVE tensor_scalar is_ge,mult; bf16 out)
    pad/halo columns of H' are set to 1.0 so that with T = conv(H', sign(w)),
    conv(xb) = T - S where S[co] = sum(sign(w[co,:,:])) for EVERY output col.
    t' = y - gamma = scale*T + c1,  c1 = bias - gamma - scale*S
    out = (1-beta)*relu(t') + (beta*t' + zeta)
        ACT1: v = Relu(scale*T + c1)            [per-partition scale/bias]
        ACT2: q = Identity(beta*scale*T + beta*c1 + zeta)
        DVE : out = (v * (1-beta)) + q          [scalar_tensor_tensor]
All conv arithmetic is exact (integer-valued products/sums in fp32 PSUM).
"""

import sys

if "/opt/trn_rl_repo" not in sys.path:
    sys.path.insert(0, "/opt/trn_rl_repo")

import numpy as np
import ml_dtypes

import concourse.bacc as bacc
import concourse.mybir as mybir
import concourse.tile as tile
from concourse.bass_utils import run_bass_kernel_spmd

P = 128          # SBUF partitions = 2 images x 64 channels
CH = 64          # channels
KTAPS = 3        # conv taps
CHUNK = 512      # PSUM bank = 512 fp32 -> matmul free dim
TW = 2048        # output columns per tile (4 PSUM banks)
L_FULL = 65536
N_CORES = 8
B_FULL = 16


def build_nc(
    L: int,
    tw: int = TW,
    repeats: int = 1,
    xbufs: int = 3,
    ebufs: int = 2,
    pbufs: int = 2,
    dsplit: int = 1,
):
    """Build the per-core Bass program for shard [P, L].

    repeats > 1 re-runs the whole body (idempotent) for marginal-cost timing.
    """
    assert L % tw == 0 and tw % CHUNK == 0
    n_tiles = L // tw
    assert n_tiles >= 2
    n_chunks = tw // CHUNK
    f32 = mybir.dt.float32
    bf16 = mybir.dt.bfloat16

    nc = bacc.Bacc("TRN2", target_bir_lowering=False, debug=False)
    x = nc.dram_tensor("x", [P, L], f32, kind="ExternalInput").ap()
    w = nc.dram_tensor("w", [KTAPS, P, P], bf16, kind="ExternalInput").ap()
    alpha2 = nc.dram_tensor("alpha2", [P, 1], f32, kind="ExternalInput").ap()
    relu_scale = nc.dram_tensor("relu_scale", [P, 1], f32, kind="ExternalInput").ap()
    relu_bias = nc.dram_tensor("relu_bias", [P, 1], f32, kind="ExternalInput").ap()
    id_scale = nc.dram_tensor("id_scale", [P, 1], f32, kind="ExternalInput").ap()
    id_bias = nc.dram_tensor("id_bias", [P, 1], f32, kind="ExternalInput").ap()
    ombeta = nc.dram_tensor("ombeta", [P, 1], f32, kind="ExternalInput").ap()
    y = nc.dram_tensor("y", [P, L], f32, kind="ExternalOutput").ap()

    xw = tw + 2  # input tile width incl. 1-col halo each side

    with tile.TileContext(nc) as tc:
        with (
            tc.tile_pool(name="const", bufs=1) as cpool,
            tc.tile_pool(name="xin", bufs=xbufs) as xpool,
            tc.tile_pool(name="hp", bufs=xbufs) as hpool,
            tc.tile_pool(name="eps", bufs=ebufs) as epool,
            tc.tile_pool(name="psum", bufs=pbufs, space="PSUM") as ppool,
        ):
            w_t = cpool.tile([P, KTAPS, P], bf16)
            for k in range(KTAPS):
                nc.sync.dma_start(out=w_t[:, k, :], in_=w[k])
            a_t = cpool.tile([P, 1], f32)
            rs_t = cpool.tile([P, 1], f32)
            rb_t = cpool.tile([P, 1], f32)
            is_t = cpool.tile([P, 1], f32)
            ib_t = cpool.tile([P, 1], f32)
            ob_t = cpool.tile([P, 1], f32)
            nc.sync.dma_start(out=a_t[:], in_=alpha2[:])
            nc.sync.dma_start(out=rs_t[:], in_=relu_scale[:])
            nc.sync.dma_start(out=rb_t[:], in_=relu_bias[:])
            nc.sync.dma_start(out=is_t[:], in_=id_scale[:])
            nc.sync.dma_start(out=ib_t[:], in_=id_bias[:])
            nc.sync.dma_start(out=ob_t[:], in_=ombeta[:])

            def dma_in_split(x_t, dst_lo, src_lo, width):
                """DMA x[:, src_lo:src_lo+width] -> x_t[:, dst_lo:...], split
                into dsplit pieces (finer DMAs mix better with the output
                stream on HBM)."""
                step = -(-width // dsplit)
                for s in range(0, width, step):
                    w = min(step, width - s)
                    nc.sync.dma_start(
                        out=x_t[:, dst_lo + s : dst_lo + s + w],
                        in_=x[:, src_lo + s : src_lo + s + w],
                    )

            for i in range(n_tiles * repeats):
                i = i % n_tiles
                base = i * tw
                x_t = xpool.tile([P, xw], f32)
                h_t = hpool.tile([P, xw], bf16)
                # load x tile (halo col j maps to x col base-1+j), rsign it
                if i == 0:
                    dma_in_split(x_t, 1, 0, tw + 1)
                    nc.vector.memset(h_t[:, 0:1], 1.0)
                    nc.vector.tensor_scalar(
                        out=h_t[:, 1:xw], in0=x_t[:, 1:xw],
                        scalar1=a_t[:], scalar2=2.0,
                        op0=mybir.AluOpType.is_ge, op1=mybir.AluOpType.mult,
                    )
                elif i == n_tiles - 1:
                    dma_in_split(x_t, 0, base - 1, tw + 1)
                    nc.vector.memset(h_t[:, xw - 1 : xw], 1.0)
                    nc.vector.tensor_scalar(
                        out=h_t[:, 0 : xw - 1], in0=x_t[:, 0 : xw - 1],
                        scalar1=a_t[:], scalar2=2.0,
                        op0=mybir.AluOpType.is_ge, op1=mybir.AluOpType.mult,
                    )
                else:
                    dma_in_split(x_t, 0, base - 1, tw + 2)
                    nc.vector.tensor_scalar(
                        out=h_t[:], in0=x_t[:],
                        scalar1=a_t[:], scalar2=2.0,
                        op0=mybir.AluOpType.is_ge, op1=mybir.AluOpType.mult,
                    )

                ps = ppool.tile([P, tw], f32)
                for c in range(n_chunks):
                    for k in range(KTAPS):
                        nc.tensor.matmul(
                            ps[:, c * CHUNK : (c + 1) * CHUNK],
                            w_t[:, k, :],
                            h_t[:, c * CHUNK + k : c * CHUNK + k + CHUNK],
                            start=(k == 0),
                            stop=(k == KTAPS - 1),
                        )

                v_t = epool.tile([P, tw], f32, tag="v")
                q_t = epool.tile([P, tw], f32, tag="q")
                o_t = epool.tile([P, tw], f32, tag="o")
                nc.scalar.activation(
                    out=v_t[:], in_=ps[:],
                    func=mybir.ActivationFunctionType.Relu,
                    bias=rb_t[:], scale=rs_t[:],
                )
                nc.scalar.activation(
                    out=q_t[:], in_=ps[:],
                    func=mybir.ActivationFunctionType.Identity,
                    bias=ib_t[:], scale=is_t[:],
                )
                nc.vector.scalar_tensor_tensor(
                    out=o_t[:], in0=v_t[:], scalar=ob_t[:], in1=q_t[:],
                    op0=mybir.AluOpType.mult, op1=mybir.AluOpType.add,
                )
                ostep = tw // dsplit
                for s in range(0, tw, ostep):
                    nc.sync.dma_start(
                        out=y[:, base + s : base + s + ostep],
                        in_=o_t[:, s : s + ostep],
                    )
    nc.compile()
    return nc


def host_prep(alpha, weight, weight_scale, bias, beta, gamma, zeta):
    """Host-side parameter folding. Returns dict of small device inputs."""
    al = np.asarray(alpha, np.float32).reshape(CH)
    sc = np.asarray(weight_scale, np.float32).reshape(CH)
    bi = np.asarray(bias, np.float32).reshape(CH)
    be = np.asarray(beta, np.float32).reshape(CH)
    ga = np.asarray(gamma, np.float32).reshape(CH)
    ze = np.asarray(zeta, np.float32).reshape(CH)
    wgt = np.asarray(weight, np.float32)  # [CH, CH, KTAPS]

    sgn = np.sign(wgt).astype(np.float32)
    s_all = sgn.sum(axis=(1, 2)).astype(np.float32)  # [CH] integer-valued

    # Block-diagonal lhsT per tap: [p_in, p_out] with two [ci, co] blocks.
    w_np = np.zeros((KTAPS, P, P), dtype=ml_dtypes.bfloat16)
    for k in range(KTAPS):
        tk = sgn[:, :, k].T.astype(ml_dtypes.bfloat16)  # [ci, co]
        w_np[k, :CH, :CH] = tk
        w_np[k, CH:, CH:] = tk

    c1 = (bi - ga - sc * s_all).astype(np.float32)

    def vec(v):
        return np.tile(v.astype(np.float32), 2).reshape(P, 1)

    return {
        "w": w_np,
        "alpha2": vec(al),
        "relu_scale": vec(sc),
        "relu_bias": vec(c1),
        "id_scale": vec(be * sc),
        "id_bias": vec(be * c1 + ze),
        "ombeta": vec(1.0 - be),
    }


def kernel(x, alpha, weight, weight_scale, bias, beta, gamma, zeta):
    x = np.asarray(x, np.float32)
    B, Cin, L = x.shape
    assert (B, Cin, L) == (B_FULL, CH, L_FULL), (B, Cin, L)

    params = host_prep(alpha, weight, weight_scale, bias, beta, gamma, zeta)
    nc = build_nc(L)

    shards = np.ascontiguousarray(x.reshape(N_CORES, P, L))
    in_maps = [dict(params, x=shards[i]) for i in range(N_CORES)]
    res = run_bass_kernel_spmd(nc, in_maps, core_ids=list(range(N_CORES)))
    out = np.stack([res.results[i]["y"] for i in range(N_CORES)])
    return out.reshape(B, CH, L).astype(np.float32)
